# revision 19
# baseline (speedup 1.0000x reference)
"""Single-core fp8 kernel with column-major [128,6] activation layout.

Same math as the baseline (single-token GEMV chain; see kernel.py), but the
residual stream u and all LayerNorm/elementwise work live in column-major
[128, 6] tiles (vec[128c+p] = tile[p,c]) so vector ops touch 128 lanes
instead of 1.  GEMVs keep the moving-weights 4-column-group structure;
flat GEMV outputs are transposed back to cm with K=1 matmuls (to_cm).
Weights are fp8 e3m4 scaled by 64; descales fold into existing scalars.
"""

import numpy as np
from contextlib import ExitStack

import concourse.bass as bass
import concourse.tile as tile
from concourse import bacc, mybir
from concourse.bass_utils import run_bass_kernel_spmd

E = 768
EC = 6
HID = 3072
CLS = 1000
L = 12
EPS = 1e-5
WS = 64.0
ISC = 1.0 / WS
INV_SQRT_E = 1.0 / float(np.sqrt(768.0))
DT = mybir.dt.float32
F8 = mybir.dt.float8e3
AX = mybir.AxisListType
OP = mybir.AluOpType
ACT = mybir.ActivationFunctionType
Q = 192      # quarter of a 768-wide GEMV output (4 col groups)
QC = 250     # quarter of the 1000-wide classifier output


def build_program(wdt=F8):
    nc = bacc.Bacc()
    inp = {}

    def din(name, shape, dt=DT):
        t = nc.dram_tensor(name, list(shape), dt, kind="ExternalInput")
        inp[name] = t
        return t

    for l in range(L):
        for c in range(2):
            din(f"wv{c}_{l}", (128, 3 * E), wdt)
            din(f"wt{c}_{l}", (128, 3 * E), wdt)
        for c in range(6):
            din(f"w1{c}_{l}", (128, HID), wdt)
        for c in range(6):
            din(f"w2{c}_{l}", (128, 4 * E), wdt)
        din(f"veccm{l}", (128, 5 * EC))      # ln1_s, ln1_b, ln2_s, ln2_b, b2 (cm)
        din(f"b1cm{l}", (128, 24))
    for c in range(6):
        din(f"wc1{c}", (128, HID), wdt)
    for c in range(8):
        din(f"wc2{c}", (128, 3 * CLS), wdt)
    din("fveccm", (128, 2 * EC))             # lnf_s, lnf_b (cm)
    din("bc2f", (1, CLS))
    din("bc1cm", (128, 24))
    din("u0cm", (128, EC))

    out_t = nc.dram_tensor("out", [1, CLS], DT, kind="ExternalOutput")

    with ExitStack() as ctx:
        tc = ctx.enter_context(tile.TileContext(nc))
        wsm = ctx.enter_context(tc.tile_pool(name="wsm", bufs=4))
        wbg = ctx.enter_context(tc.tile_pool(name="wbg", bufs=16))
        vp = ctx.enter_context(tc.tile_pool(name="vp", bufs=3))
        pers = ctx.enter_context(tc.tile_pool(name="pers", bufs=1))
        wk = ctx.enter_context(tc.tile_pool(name="wk", bufs=2))
        ps_at = ctx.enter_context(tc.tile_pool(name="ps_at", bufs=1, space="PSUM"))
        ps_m = ctx.enter_context(tc.tile_pool(name="ps_m", bufs=3, space="PSUM"))
        ps_t = ctx.enter_context(tc.tile_pool(name="ps_t", bufs=1, space="PSUM"))

        epst = pers.tile([1, 1], DT)
        nc.vector.memset(epst[:], EPS)
        onet = pers.tile([1, 1], DT)
        nc.vector.memset(onet[:], 1.0)
        ones_col = pers.tile([128, 1], DT)
        nc.vector.memset(ones_col[:], 1.0)
        ones_row = pers.tile([1, 128], DT)
        nc.vector.memset(ones_row[:], 1.0)
        u = pers.tile([128, EC], DT)
        h = pers.tile([128, EC], DT)

        def layer_norm_cm(x_ap, g_ap, b_ap, out_q, out_f32=None, tag=""):
            """LN over cm [128,6]: stats via matmul reduce + bcast."""
            st = wk.tile([128, 2], DT, tag="lnst", name=f"st{tag}")
            sq = wk.tile([128, 1], DT, tag="lnsq", name=f"sq{tag}")
            scr = wk.tile([128, EC], DT, tag="lnscr", name=f"scr{tag}")
            scr2 = wk.tile([128, EC], DT, tag="lnscr2", name=f"scr2{tag}")
            nc.vector.tensor_scalar(
                out=scr[:], in0=x_ap, scalar1=1.0 / E, scalar2=None,
                op0=OP.mult, op1=OP.add, accum_out=st[:, 0:1])
            nc.vector.scalar_tensor_tensor(
                out=scr2[:], in0=x_ap, scalar=1.0, in1=x_ap,
                op0=OP.mult, op1=OP.mult, accum_out=st[:, 1:2])
            psS = ps_t.tile([1, 2], DT, tag="psS", name=f"psS{tag}")
            nc.tensor.matmul(psS[:], ones_col[:], st[:], start=True, stop=True)
            ss = wk.tile([1, 2], DT, tag="lnss", name=f"ss{tag}")
            nc.scalar.copy(out=ss[:], in_=psS[:])
            mr = wk.tile([1, 2], DT, tag="lnmr", name=f"mr{tag}")
            nc.scalar.copy(out=mr[0:1, 0:1], in_=ss[0:1, 0:1])
            v = wk.tile([1, 1], DT, tag="lnv", name=f"v{tag}")
            nc.vector.tensor_scalar(
                out=v[:], in0=ss[0:1, 1:2], scalar1=1.0 / E, scalar2=None,
                op0=OP.mult)
            m2 = wk.tile([1, 1], DT, tag="lnm2", name=f"m2{tag}")
            nc.vector.tensor_scalar(
                out=m2[:], in0=ss[0:1, 0:1], scalar1=ss[0:1, 0:1], scalar2=None,
                op0=OP.mult)
            nc.vector.tensor_sub(v[:], v[:], m2[:])
            sd = wk.tile([1, 1], DT, tag="lnsd", name=f"sd{tag}")
            nc.scalar.activation(out=sd[:], in_=v[:], func=ACT.Sqrt, bias=epst[:])
            nc.vector.reciprocal(mr[0:1, 1:2], sd[:])
            psB1 = ps_t.tile([128, 2], DT, tag="psB1", name=f"psB1{tag}")
            nc.tensor.matmul(psB1[:], ones_row[:], mr[:], start=True, stop=True)
            mrb = wk.tile([128, 2], DT, tag="lnmrb", name=f"mrb{tag}")
            nc.scalar.copy(out=mrb[:], in_=psB1[:])
            t6 = wk.tile([128, EC], DT, tag="lnt6", name=f"t6{tag}")
            nc.vector.scalar_tensor_tensor(
                out=t6[:], in0=x_ap, scalar=mrb[:, 0:1], in1=g_ap,
                op0=OP.subtract, op1=OP.mult)
            if out_f32 is not None:
                nc.vector.scalar_tensor_tensor(
                    out=out_f32, in0=t6[:], scalar=mrb[:, 1:2], in1=b_ap,
                    op0=OP.mult, op1=OP.add)
                nc.scalar.copy(out=out_q, in_=out_f32)
            else:
                nc.vector.scalar_tensor_tensor(
                    out=out_q, in0=t6[:], scalar=mrb[:, 1:2], in1=b_ap,
                    op0=OP.mult, op1=OP.add)

        def to_cm(flat_ap, n_seg, tag, dt=None, scale=None, bias_ap=None):
            """[1, 128*n_seg] SBUF flat -> [128, n_seg] cm tile."""
            ps = ps_t.tile([128, n_seg], DT, tag="tps", name=f"tps{tag}")
            for s in range(n_seg):
                nc.tensor.matmul(
                    ps[:, s:s + 1], flat_ap[0:1, 128 * s:128 * (s + 1)],
                    onet[:], start=True, stop=True)
            cm = wk.tile([128, n_seg], dt or DT, tag=tag)
            if bias_ap is not None:
                nc.vector.scalar_tensor_tensor(
                    out=cm[:], in0=ps[:], scalar=scale or 1.0, in1=bias_ap,
                    op0=OP.mult, op1=OP.add)
            elif scale is not None:
                nc.vector.tensor_scalar(out=cm[:], in0=ps[:], scalar1=scale,
                                        scalar2=None, op0=OP.mult)
            else:
                nc.vector.tensor_copy(out=cm[:], in_=ps[:])
            return cm

        def from_cm(cm_ap, n_seg, tag, eng=None):
            """[128, n_seg] cm -> [1, 128*n_seg] flat f8 (for GEMV lhsT)."""
            # transpose matmul: out[1, 128] per seg: lhsT = cm col [128,1]
            ps = ps_t.tile([1, 128 * n_seg], DT, tag="fps", name=f"fps{tag}")
            for s in range(n_seg):
                nc.tensor.matmul(
                    ps[0:1, 128 * s:128 * (s + 1)], cm_ap[:, s:s + 1],
                    ones_row[:], start=True, stop=True, is_transpose=True)
            fl = wk.tile([1, 128 * n_seg], F8, tag=tag)
            nc.scalar.copy(out=fl[:], in_=ps[:])
            return fl

        def tocm4_192(src_ps, tag, scale=None, bias_ap=None):
            """psum 4x[1,192] rows (part 32g) -> cm [128,6], 4-row-parallel."""
            f4 = wk.tile([128, Q], DT, tag=f"f4{tag}", name=f"f4{tag}")
            for g in range(4):
                eng = nc.vector if g % 2 == 0 else nc.scalar
                if g % 2 == 0:
                    nc.vector.tensor_copy(out=f4[32 * g:32 * g + 1, :],
                                          in_=src_ps[32 * g:32 * g + 1, 0:Q])
                else:
                    nc.scalar.copy(out=f4[32 * g:32 * g + 1, :],
                                   in_=src_ps[32 * g:32 * g + 1, 0:Q])
            ps = ps_t.tile([128, EC], DT, tag="tps", name=f"tps{tag}")
            for st_ in range(EC):
                lo, hi = 128 * st_, 128 * st_ + 128
                g0, o0 = lo // Q, lo % Q
                if o0 + 128 <= Q:
                    nc.tensor.matmul(
                        ps[:, st_:st_ + 1], f4[32 * g0:32 * g0 + 1, o0:o0 + 128],
                        ones_col[32 * g0:32 * g0 + 1, 0:1], start=True, stop=True,
                        tile_position=(32 * g0, 0), skip_group_check=True)
                else:
                    n0 = Q - o0
                    nc.tensor.matmul(
                        ps[0:n0, st_:st_ + 1], f4[32 * g0:32 * g0 + 1, o0:Q],
                        ones_col[32 * g0:32 * g0 + 1, 0:1], start=True, stop=True,
                        tile_position=(32 * g0, 0), skip_group_check=True)
                    g1 = g0 + 1
                    nc.tensor.matmul(
                        ps[n0:128, st_:st_ + 1],
                        f4[32 * g1:32 * g1 + 1, 0:128 - n0],
                        ones_col[32 * g1:32 * g1 + 1, 0:1], start=True, stop=True,
                        tile_position=(32 * g1, n0), skip_group_check=True)
            cm = wk.tile([128, EC], DT, tag=f"cm{tag}")
            if bias_ap is not None:
                nc.vector.scalar_tensor_tensor(
                    out=cm[:], in0=ps[:], scalar=scale or 1.0, in1=bias_ap,
                    op0=OP.mult, op1=OP.add)
            elif scale is not None:
                nc.vector.tensor_scalar(out=cm[:], in0=ps[:], scalar1=scale,
                                        scalar2=None, op0=OP.mult)
            else:
                nc.vector.tensor_copy(out=cm[:], in_=ps[:])
            return cm

        def tocm4_512(psC_, psD_, tag, scale, bias_ap):
            """6 psum rows of [1,512] -> cm [128,24], 4-row-parallel."""
            gf4 = wk.tile([128, 768], DT, tag="gf4", name=f"gf4{tag}")
            for g in range(4):
                if g % 2 == 0:
                    nc.vector.tensor_copy(out=gf4[32 * g:32 * g + 1, 0:384],
                                          in_=psC_[32 * g:32 * g + 1, 0:384])
                    nc.scalar.copy(out=gf4[32 * g:32 * g + 1, 384:768],
                                   in_=psD_[32 * g:32 * g + 1, 0:384])
                else:
                    nc.scalar.copy(out=gf4[32 * g:32 * g + 1, 0:384],
                                   in_=psC_[32 * g:32 * g + 1, 0:384])
                    nc.vector.tensor_copy(out=gf4[32 * g:32 * g + 1, 384:768],
                                          in_=psD_[32 * g:32 * g + 1, 0:384])
            ps = ps_t.tile([128, 24], DT, tag="tps", name=f"tps{tag}")
            for st_ in range(24):
                g, off = st_ // 6, 128 * (st_ % 6)
                nc.tensor.matmul(
                    ps[:, st_:st_ + 1], gf4[32 * g:32 * g + 1, off:off + 128],
                    ones_col[32 * g:32 * g + 1, 0:1], start=True, stop=True,
                    tile_position=(32 * g, 0), skip_group_check=True)
            cm = wk.tile([128, 24], DT, tag="gcm32")
            nc.vector.scalar_tensor_tensor(
                out=cm[:], in0=ps[:], scalar=scale, in1=bias_ap,
                op0=OP.mult, op1=OP.add)
            return cm

        def mm_ct(pt, row, lhs_col, rhs_ap, start, stop):
            nc.tensor.matmul(
                pt[32 * row:32 * row + 1, 0:rhs_ap.shape[-1]], lhs_col, rhs_ap,
                start=start, stop=stop, tile_position=(0, 32 * row),
                skip_group_check=True)

        def load_attn_vec(l):
            vec = vp.tile([128, 5 * EC], DT, tag="vec", name=f"vec{l}_t")
            nc.sync.dma_start(out=vec[:], in_=inp[f"veccm{l}"][:, :])
            b1cm = vp.tile([128, 24], DT, tag="b1cm", name=f"b1cm{l}_t")
            nc.sync.dma_start(out=b1cm[:], in_=inp[f"b1cm{l}"][:, :])
            wv_, wt_ = [], []
            for c in range(2):
                wvt = wsm.tile([128, 3 * E], F8, tag="wv", name=f"wv{c}_{l}_t")
                nc.sync.dma_start(out=wvt[:], in_=inp[f"wv{c}_{l}"][:, :])
                wv_.append(wvt)
                wtt = wsm.tile([128, 3 * E], F8, tag="wt", name=f"wt{c}_{l}_t")
                nc.sync.dma_start(out=wtt[:], in_=inp[f"wt{c}_{l}"][:, :])
                wt_.append(wtt)
            return wv_, wt_, vec, b1cm

        nc.sync.dma_start(out=u[:], in_=inp["u0cm"][:, :])

        nxt = load_attn_vec(0)
        for l in range(L):
            wv_, wt_, vec, b1cm = nxt
            w1c_ = []
            for c in range(6):
                wti = wbg.tile([128, HID], F8, tag="wb")
                nc.sync.dma_start(out=wti[:], in_=inp[f"w1{c}_{l}"][:, :])
                w1c_.append(wti)
            w2c_ = []
            for c in range(6):
                wti = wbg.tile([128, 4 * E], F8, tag="wb")
                nc.sync.dma_start(out=wti[:], in_=inp[f"w2{c}_{l}"][:, :])
                w2c_.append(wti)

            # ---- LN1 (cm) -> h (f32) + hq (f8 flat lhs) ----
            hq8 = wk.tile([128, EC], F8, tag="hq8", name=f"hq8{l}")
            layer_norm_cm(u[:], vec[:, 0:EC], vec[:, EC:2 * EC], hq8[:],
                          out_f32=h[:], tag=f"a{l}")

            # ---- a = h@Wv, t = h@Wtheta (4-way col-tiled, moving weights) ----
            psA = ps_at.tile([128, 512], DT, tag="pa")
            psB = ps_at.tile([128, 512], DT, tag="pb")
            for s in range(EC):
                st, sp = (s == 0), (s == EC - 1)
                lhs = hq8[:, s:s + 1]
                c, sl = s // 3, s % 3
                for g in range(4):
                    mm_ct(psA, g, lhs, wv_[c][:, sl * E + g * Q: sl * E + (g + 1) * Q], st, sp)
                for g in range(4):
                    mm_ct(psB, g, lhs, wt_[c][:, sl * E + g * Q: sl * E + (g + 1) * Q], st, sp)

            if l + 1 < L:
                nxt = load_attn_vec(l + 1)

            acm = tocm4_192(psA, "acm")
            tcm = tocm4_192(psB, "tcm")

            # sval = (h . t)*ISC*INV_SQRT_E ; c0 = (1+sval)*ISC
            hm = wk.tile([128, EC], DT, tag="hm", name=f"hm{l}")
            sv = wk.tile([128, 1], DT, tag="sv", name=f"sv{l}")
            nc.vector.tensor_mul(hm[:], h[:], tcm[:])
            nc.vector.tensor_scalar(
                out=hm[:], in0=hm[:], scalar1=1.0, scalar2=None,
                op0=OP.mult, op1=OP.add, accum_out=sv[:])
            psSV = ps_t.tile([1, 2], DT, tag="psS", name=f"psSV{l}")
            nc.tensor.matmul(psSV[0:1, 0:1], ones_col[:], sv[:], start=True, stop=True)
            c0s = wk.tile([1, 1], DT, tag="c0s", name=f"c0s{l}")
            nc.vector.tensor_scalar(
                out=c0s[:], in0=psSV[0:1, 0:1], scalar1=ISC * INV_SQRT_E, scalar2=1.0,
                op0=OP.mult, op1=OP.add)
            psC0 = ps_t.tile([128, 2], DT, tag="psB1", name=f"psC0{l}")
            nc.tensor.matmul(psC0[:, 0:1], ones_row[:], c0s[:], start=True, stop=True)
            # u = h + a * c0 * ISC  (a is 64x)
            c0b = wk.tile([128, 1], DT, tag="c0b", name=f"c0b{l}")
            nc.vector.tensor_scalar(
                out=c0b[:], in0=psC0[:, 0:1], scalar1=ISC, scalar2=None, op0=OP.mult)
            nc.vector.scalar_tensor_tensor(
                out=u[:], in0=acm[:], scalar=c0b[:, 0:1], in1=h[:],
                op0=OP.mult, op1=OP.add)

            # ---- LN2 -> h2q (f8) ----
            h2q = wk.tile([128, EC], F8, tag="h2q", name=f"h2q{l}")
            layer_norm_cm(u[:], vec[:, 2 * EC:3 * EC], vec[:, 3 * EC:4 * EC],
                          h2q[:], tag=f"b{l}")

            # ---- m1 = h2@W1: 6 n-tiles of 512 ----
            psC = ps_m.tile([128, 512], DT, tag="m")
            psD = ps_m.tile([128, 512], DT, tag="m")
            for s in range(EC):
                st, sp = (s == 0), (s == EC - 1)
                lhs = h2q[:, s:s + 1]
                wsrc = w1c_[s]
                for g in range(4):
                    mm_ct(psC, g, lhs, wsrc[:, 768 * g: 768 * g + 384], st, sp)
                    mm_ct(psD, g, lhs, wsrc[:, 768 * g + 384: 768 * g + 768], st, sp)
            gcm32 = tocm4_512(psC, psD, f"g{l}", ISC, b1cm[:])
            gcm = wk.tile([128, 24], F8, tag="gcm")
            nc.scalar.activation(out=gcm[:], in_=gcm32[:], func=ACT.Gelu)

            # ---- m2 = g@W2 (4x192 col groups); u += m2*ISC + b2 ----
            psE = ps_m.tile([128, 512], DT, tag="m")
            for s in range(24):
                st, sp = (s == 0), (s == 23)
                lhs = gcm[:, s:s + 1]
                wsrc = w2c_[s // 4]
                sl = s % 4
                for g in range(4):
                    mm_ct(psE, g, lhs, wsrc[:, sl * E + g * Q: sl * E + (g + 1) * Q],
                          st, sp)
            mcm = tocm4_192(psE, "mcm", scale=ISC, bias_ap=vec[:, 4 * EC:5 * EC])
            nc.vector.tensor_add(u[:], u[:], mcm[:])

        # ---- classifier ----
        fvec = vp.tile([128, 2 * EC], DT, tag="vec", name="fvec_t")
        nc.sync.dma_start(out=fvec[:], in_=inp["fveccm"][:, :])
        bc2f = vp.tile([1, CLS], DT, tag="bc2f", name="bc2f_t")
        nc.sync.dma_start(out=bc2f[:], in_=inp["bc2f"][:, :])
        bc1cm = vp.tile([128, 24], DT, tag="b1cm", name="bc1cm_t")
        nc.sync.dma_start(out=bc1cm[:], in_=inp["bc1cm"][:, :])

        cfq = wk.tile([128, EC], F8, tag="hq8", name="cfq")
        layer_norm_cm(u[:], fvec[:, 0:EC], fvec[:, EC:2 * EC], cfq[:], tag="f")

        wc1c_ = []
        for c in range(6):
            wti = wbg.tile([128, HID], F8, tag="wb")
            nc.sync.dma_start(out=wti[:], in_=inp[f"wc1{c}"][:, :])
            wc1c_.append(wti)
        psC = ps_m.tile([128, 512], DT, tag="m")
        psD = ps_m.tile([128, 512], DT, tag="m")
        for s in range(EC):
            st, sp = (s == 0), (s == EC - 1)
            lhs = cfq[:, s:s + 1]
            wsrc = wc1c_[s]
            for g in range(4):
                mm_ct(psC, g, lhs, wsrc[:, 768 * g: 768 * g + 384], st, sp)
                mm_ct(psD, g, lhs, wsrc[:, 768 * g + 384: 768 * g + 768], st, sp)
        g2cm32 = tocm4_512(psC, psD, "gc", ISC, bc1cm[:])
        g2cm = wk.tile([128, 24], F8, tag="gcm")
        nc.scalar.activation(out=g2cm[:], in_=g2cm32[:], func=ACT.Gelu)

        wc2 = []
        for c in range(8):
            w = wbg.tile([128, 3 * CLS], F8, tag="wb")
            nc.sync.dma_start(out=w[:], in_=inp[f"wc2{c}"][:, :])
            wc2.append(w)
        psF = ps_m.tile([128, 512], DT, tag="m")
        for s in range(24):
            st, sp = (s == 0), (s == 23)
            lhs = g2cm[:, s:s + 1]
            wsrc = wc2[s // 3]
            sl = s % 3
            for g in range(4):
                mm_ct(psF, g, lhs, wsrc[:, sl * CLS + g * QC: sl * CLS + (g + 1) * QC],
                      st, sp)
        lg = wk.tile([1, CLS], DT, tag="lg")
        for g in range(4):
            nc.vector.tensor_scalar(
                out=lg[0:1, g * QC:(g + 1) * QC], in0=psF[32 * g:32 * g + 1, 0:QC],
                scalar1=ISC, scalar2=None, op0=OP.mult)
        nc.vector.tensor_add(lg[:], lg[:], bc2f[:])

        # log_softmax (flat, once)
        mx = wk.tile([1, 1], DT, tag="mx")
        nc.vector.reduce_max(mx[:], lg[:], axis=AX.X)
        sh = wk.tile([1, CLS], DT, tag="sh")
        nc.vector.tensor_scalar(
            out=sh[:], in0=lg[:], scalar1=mx[:], scalar2=None, op0=OP.subtract)
        se = wk.tile([1, 1], DT, tag="se")
        nc.scalar.activation(out=lg[:], in_=sh[:], func=ACT.Exp, accum_out=se[:])
        lse = wk.tile([1, 1], DT, tag="lse")
        nc.scalar.activation(out=lse[:], in_=se[:], func=ACT.Ln)
        nc.vector.tensor_scalar(
            out=sh[:], in0=sh[:], scalar1=lse[:], scalar2=None, op0=OP.subtract)
        nc.sync.dma_start(out=out_t[:, :], in_=sh[:])

    nc.compile()
    return nc


def _cm(vec, ncols):
    v = np.asarray(vec, np.float32)
    return np.ascontiguousarray(v.reshape(ncols, 128).T)


def prep_inputs(inputs):
    import ml_dtypes
    f32 = lambda x: np.ascontiguousarray(np.asarray(x, dtype=np.float32))
    fw = lambda x: np.ascontiguousarray(
        (np.asarray(x, np.float32) * WS).astype(ml_dtypes.float8_e3m4))
    m = {}
    Wv, Wt = inputs["Wv"], inputs["Wtheta"]
    W1, W2 = inputs["W1"], inputs["W2"]
    for l in range(L):
        wv = np.asarray(Wv[l]).reshape(6, 128, E).transpose(1, 0, 2)
        wt = np.asarray(Wt[l]).reshape(6, 128, E).transpose(1, 0, 2)
        for c in range(2):
            m[f"wv{c}_{l}"] = fw(wv[:, 3 * c:3 * c + 3].reshape(128, 3 * E))
            m[f"wt{c}_{l}"] = fw(wt[:, 3 * c:3 * c + 3].reshape(128, 3 * E))
        w1 = np.asarray(W1[l]).reshape(6, 128, HID).transpose(1, 0, 2)
        for c in range(6):
            m[f"w1{c}_{l}"] = fw(w1[:, c].reshape(128, HID))
        w2 = np.asarray(W2[l]).reshape(24, 128, E).transpose(1, 0, 2)
        for c in range(6):
            m[f"w2{c}_{l}"] = fw(w2[:, 4 * c:4 * c + 4].reshape(128, 4 * E))
        m[f"veccm{l}"] = np.concatenate([
            _cm(inputs["ln1_s"][l], EC), _cm(inputs["ln1_b"][l], EC),
            _cm(inputs["ln2_s"][l], EC), _cm(inputs["ln2_b"][l], EC),
            _cm(inputs["b2"][l], EC)], axis=1)
        m[f"b1cm{l}"] = f32(np.asarray(inputs["b1"][l]).reshape(24, 128).T)
    wc1 = np.asarray(inputs["Wc1"]).reshape(6, 128, HID).transpose(1, 0, 2)
    for c in range(6):
        m[f"wc1{c}"] = fw(wc1[:, c].reshape(128, HID))
    wc2 = np.asarray(inputs["Wc2"]).reshape(24, 128, CLS).transpose(1, 0, 2)
    for c in range(8):
        m[f"wc2{c}"] = fw(wc2[:, 3 * c:3 * c + 3].reshape(128, 3 * CLS))
    m["fveccm"] = np.concatenate([
        _cm(inputs["lnf_s"], EC), _cm(inputs["lnf_b"], EC)], axis=1)
    m["bc2f"] = f32(inputs["bc2"]).reshape(1, CLS)
    m["bc1cm"] = f32(np.asarray(inputs["bc1"]).reshape(24, 128).T)
    u0 = np.asarray(inputs["class_token"]).reshape(E) + \
        np.asarray(inputs["pos"]).reshape(-1, E)[-1]
    m["u0cm"] = _cm(u0, EC)
    return m


_CACHED = {}


def kernel(**inputs) -> np.ndarray:
    b = int(np.asarray(inputs["x"]).shape[0])
    in_map = prep_inputs(inputs)
    if "nc" not in _CACHED:
        _CACHED["nc"] = build_program()
    nc = _CACHED["nc"]
    r = run_bass_kernel_spmd(nc, [in_map], core_ids=[0])
    out = np.asarray(r.results[0]["out"]).reshape(1, CLS)
    return np.ascontiguousarray(np.broadcast_to(out, (b, CLS)).astype(np.float32))
